# revision 1
# baseline (speedup 1.0000x reference)
"""Trainium2 Bass kernel for nn_Attention_41704132444382.

Masked-linear QKV projection + 16-head attention + masked-linear output
projection, tensor-parallel over heads across 8 NeuronCores (2 heads/core).

Layout strategy (all chosen to avoid on-device transposes of activations):
  - x is passed host-transposed as xT [1024, 4096] (k on partitions).
  - Q^T, K^T computed as [128 (2 heads x 64 d), 4096 t]  (d on partitions).
  - V^T computed the same way, then PE-transposed to V [t, dv] tiles with a
    ones column appended -> PV matmul yields attnout^T AND the softmax
    denominator (colsum) in one accumulation chain (M=65).
  - Scores computed as S^T [j keys on partitions, i queries free]; exp runs
    on ScalarE directly from PSUM with the 1/32 scale folded in (no max
    subtraction needed: |scores/32| <= ~7 so exp is safe in fp32).
  - Normalization: colsum rows are PE-transpose-gathered onto partitions,
    reciprocal on DVE, PE K=1-broadcast back to [64, i], fused into the
    PSUM->SBUF evacuation of attnout^T via tensor_tensor mult.
  - Output projection consumes attnT (dv on partitions) as lhsT directly;
    per-core partial outputs are summed on host; bias applied on host.

Matmuls use float32r (fp32 storage, 1 cyc/row on PE when N>=256 vs fp32's 4).
Set BASS_ATTN_F32R=0 to fall back to plain fp32 matmuls.
"""

import os
import sys

import numpy as np

sys.path.insert(0, "/opt/trn_rl_repo")

import concourse.bass as bass
import concourse.mybir as mybir
from concourse import bacc
from concourse.masks import make_identity
from concourse.tile import TileContext

DIM = 1024
HEADS = 16
B = 2
N = 2048
T = B * N  # 4096 flattened tokens
NCORES = 8
HPC = HEADS // NCORES  # 2 heads per core
DV = HPC * 64  # 128 head-dims per core
SCALE = DIM ** (-0.5)  # 1/32

F32 = mybir.dt.float32
F32R = mybir.dt.float32r

# matmul-operand dtype: "f32r" (default), "f32", or "bf16"
MM_DTYPE_NAME = os.environ.get("BASS_ATTN_MM_DTYPE", "f32r")
DT_MM = {"f32r": mybir.dt.float32r, "f32": F32, "bf16": mybir.dt.bfloat16}[MM_DTYPE_NAME]


def build_nc():
    nc = bacc.Bacc("TRN2", target_bir_lowering=True)
    xT_d = nc.declare_dram_parameter("xT", [DIM, T], F32, isOutput=False)
    wqkvT_d = nc.declare_dram_parameter("wqkvT", [DIM, 384], F32, isOutput=False)
    mqkvT_d = nc.declare_dram_parameter("mqkvT", [DIM, 384], F32, isOutput=False)
    woT_d = nc.declare_dram_parameter("woT", [DV, DIM], F32, isOutput=False)
    moT_d = nc.declare_dram_parameter("moT", [DV, DIM], F32, isOutput=False)
    out_d = nc.declare_dram_parameter("out", [T, DIM], F32, isOutput=True)

    gt = mybir.AluOpType.is_gt
    mult = mybir.AluOpType.mult
    Exp = mybir.ActivationFunctionType.Exp

    with TileContext(nc) as tc:
        with tc.tile_pool(name="persist", bufs=1) as pp:
            wqkv_g = pp.tile([128, 8 * 384], DT_MM)  # [k-part, (kt, o)]
            wo_g = pp.tile([128, 1024], DT_MM)
            qT = pp.tile([128, 4096], DT_MM)
            kTt = pp.tile([128, 4096], DT_MM)
            v1 = pp.tile([128, 32 * 65], DT_MM)  # [t-part, (jt, dv|1)] head 1
            v2 = pp.tile([128, 32 * 65], DT_MM)
            attnT = [pp.tile([128, 2048], DT_MM, name=f"attnT{bb}") for bb in range(B)]
            ident = pp.tile([128, 128], F32)
            ones1 = pp.tile([1, 64], DT_MM)

            make_identity(nc, ident[:])
            # memset can't emit float32r; memset f32 then cast-copy
            ones_f = pp.tile([128, 64], F32)
            nc.vector.memset(ones_f[:], 1.0)
            nc.vector.tensor_copy(ones1[:], ones_f[0:1, :])
            ones32 = pp.tile([128, 32], DT_MM)
            nc.vector.tensor_copy(ones32[:], ones_f[:, 0:32])
            # ones column at slot 64 of each 65-wide block of v1/v2 (strided write);
            # V evacuations only write cols 0..63 of each block.
            for vv in (v1, v2):
                nc.vector.tensor_copy(
                    vv[:].rearrange("p (j c) -> p j c", c=65)[:, :, 64:65],
                    ones32[:].rearrange("p (j c) -> p j c", c=1),
                )

            # ---------- Phase 0: load + gate weights ----------
            with tc.tile_pool(name="wload", bufs=2) as wl:
                wraw = wl.tile([128, 8 * 384], F32, tag="w")
                mraw = wl.tile([128, 8 * 384], F32, tag="w")
                g = wl.tile([128, 8 * 384], F32, tag="g")
                nc.sync.dma_start(
                    wraw[:].rearrange("p (kt o) -> p kt o", kt=8),
                    wqkvT_d[:].rearrange("(kt p) o -> p kt o", p=128),
                )
                nc.sync.dma_start(
                    mraw[:].rearrange("p (kt o) -> p kt o", kt=8),
                    mqkvT_d[:].rearrange("(kt p) o -> p kt o", p=128),
                )
                nc.vector.tensor_scalar(g[:], mraw[:], 0.0, None, gt)
                nc.vector.tensor_tensor(wqkv_g[:], wraw[:], g[:], mult)

                wor = wl.tile([128, 1024], F32, tag="wo")
                mor = wl.tile([128, 1024], F32, tag="wo")
                go = wl.tile([128, 1024], F32, tag="go")
                nc.sync.dma_start(wor[:], woT_d[:])
                nc.sync.dma_start(mor[:], moT_d[:])
                nc.vector.tensor_scalar(go[:], mor[:], 0.0, None, gt)
                nc.vector.tensor_tensor(wo_g[:], wor[:], go[:], mult)

            # ---------- Phase 1: QKV projection (+ V^T transpose) ----------
            vT = pp.tile([128, 4096], F32)
            with (
                tc.tile_pool(name="xq", bufs=16) as xp,
                tc.tile_pool(name="qk_ps", bufs=4, space="PSUM") as qkps,
            ):
                for q in range(4):  # t-quarters of 1024
                    xq = [xp.tile([128, 1024], DT_MM, tag="xq", name=f"xq{q}_{i}") for i in range(8)]
                    dma_x = nc.sync if DT_MM == F32 else nc.gpsimd
                    for kt in range(8):
                        dma_x.dma_start(
                            xq[kt][:],
                            xT_d[kt * 128 : (kt + 1) * 128, q * 1024 : (q + 1) * 1024],
                        )
                    for ot, dest in enumerate((qT, kTt, vT)):
                        for th in range(2):  # 512-wide halves of the quarter
                            ps = qkps.tile([128, 512], F32, tag="qkps")
                            for kt in range(8):
                                nc.tensor.matmul(
                                    ps[:],
                                    wqkv_g[
                                            :,
                                            kt * 384 + ot * 128 : kt * 384 + (ot + 1) * 128,
                                        ]
                                    ,
                                    xq[kt][:, th * 512 : (th + 1) * 512],
                                    start=(kt == 0),
                                    stop=(kt == 7),
                                )
                            col = q * 1024 + th * 512
                            nc.vector.tensor_copy(dest[:, col : col + 512], ps[:])


            # ---------- Phase 2: attention ----------
            with (
                tc.tile_pool(name="es", bufs=10) as ep,
                tc.tile_pool(name="small", bufs=4) as sp,
                tc.tile_pool(name="unorm", bufs=4) as up,
                tc.tile_pool(name="os", bufs=6) as osp,
                tc.tile_pool(name="s_ps", bufs=4, space="PSUM") as sps,
                tc.tile_pool(name="pv_ps", bufs=2, space="PSUM") as pvps,
            ):
                # V^T [dv, t] -> V [t, dv] via PE transpose at the head of
                # phase 2 (s-tag psum slots) so the PE has dense work across
                # the phase boundary
                for jt in range(32):
                    ptv = sps.tile([128, 128], F32, tag="s", name=f"ptv{jt}")
                    nc.tensor.transpose(ptv[:], vT[:, jt * 128 : (jt + 1) * 128], ident[:])
                    nc.vector.tensor_copy(v1[:, jt * 65 : jt * 65 + 64], ptv[:, 0:64])
                    nc.vector.tensor_copy(v2[:, jt * 65 : jt * 65 + 64], ptv[:, 64:128])

                def emit_po(pb, pib, tt):
                    # out-projection matmuls for an already-normalized block
                    for oh in range(2):
                        tg = pib * 8 + tt
                        po = sps.tile([128, 512], F32, tag="s", name=f"po{pb}_{pib}_{tt}_{oh}")
                        nc.tensor.matmul(
                            po[:],
                            attnT[pb][:, tg * 128 : (tg + 1) * 128],
                            wo_g[:, oh * 512 : (oh + 1) * 512],
                            start=True,
                            stop=True,
                        )
                        ob = osp.tile([128, 512], F32, tag="ob", name=f"ob{pb}_{pib}_{tt}_{oh}")
                        if (tt + oh) % 2 == 0:
                            nc.vector.tensor_copy(ob[:], po[:])
                        else:
                            nc.scalar.copy(ob[:], po[:])
                        row = pb * 2048 + tg * 128
                        nc.sync.dma_start(
                            out_d[row : row + 128, oh * 512 : (oh + 1) * 512], ob[:]
                        )

                prev_block = None
                for b in range(B):
                    for ib in range(2):  # 1024-wide query blocks
                        i0 = b * 2048 + ib * 1024
                        pv = [pvps.tile([65, 1024], F32, tag="pv", name=f"pv{b}_{ib}_{i}") for i in range(2)]
                        for jt in range(16):  # 128-wide key tiles
                            j0 = b * 2048 + jt * 128
                            jv = (b * 16 + jt) * 65
                            # one 1-bank psum tile per (head, i-half): 4 slots ->
                            # deeper S->exp->PV pipeline keeps the PE array dense
                            s_h = [sps.tile([128, 512], F32, tag="s", name=f"s{b}_{ib}_{jt}_{i}") for i in range(4)]
                            e_h = [ep.tile([128, 512], DT_MM, tag="e", name=f"e{b}_{ib}_{jt}_{i}") for i in range(4)]
                            for h in range(2):
                                kTl = kTt[h * 64 : (h + 1) * 64, j0 : j0 + 128]
                                for ih in range(2):
                                    st = s_h[h * 2 + ih]
                                    nc.tensor.matmul(
                                        st[:],
                                        kTl,
                                        qT[
                                            h * 64 : (h + 1) * 64,
                                            i0 + ih * 512 : i0 + (ih + 1) * 512,
                                        ],
                                        start=True,
                                        stop=True,
                                        tile_position=(h * 64, 0),
                                    )
                                    nc.scalar.activation(
                                        e_h[h * 2 + ih][:], st[:], Exp, scale=SCALE
                                    )
                            for h, vv in enumerate((v1, v2)):
                                for ih in range(2):
                                    nc.tensor.matmul(
                                        pv[h][:, ih * 512 : (ih + 1) * 512],
                                        vv[:, jv : jv + 65],
                                        e_h[h * 2 + ih][:],
                                        start=(jt == 0),
                                        stop=(jt == 15),
                                    )
                            if prev_block is not None and jt % 2 == 1:
                                emit_po(prev_block[0], prev_block[1], jt // 2)
                        # --- normalization ---
                        # colsum rows live on psum partition 64; gather each to a
                        # partition-0 [1, 1024] tile (32-aligned reads/writes only)
                        cs_h = [sp.tile([1, 1024], F32, tag="cs", name=f"cs{b}_{ib}_{i}") for i in range(2)]
                        unorm = [up.tile([64, 1024], F32, tag="un", name=f"un{b}_{ib}_{i}") for i in range(2)]
                        for h in range(2):
                            nc.vector.tensor_copy(cs_h[h][:], pv[h][64:65, :])
                            # evacuate unnormalized attnout now so the pv psum
                            # banks free early; normalize later from SBUF
                            nc.vector.tensor_copy(unorm[h][:], pv[h][0:64, :])
                        # transpose 128-wide row chunks onto partitions: col c = blk*2+h
                        pt = pvps.tile([128, 16], F32, tag="pv")
                        for h in range(2):
                            for blk in range(8):
                                nc.tensor.transpose(
                                    pt[:, (blk * 2 + h) : (blk * 2 + h) + 1],
                                    cs_h[h][0:1, blk * 128 : (blk + 1) * 128],
                                    ident[0:1, 0:1],
                                )
                        cst = sp.tile([128, 16], F32, tag="cst")
                        nc.vector.tensor_copy(cst[:], pt[:])
                        rT = sp.tile([128, 16], F32, tag="rT")
                        nc.vector.reciprocal(rT[:], cst[:])
                        # transpose each column back to a [1, 128] row at partition 0
                        r2 = [sp.tile([1, 1024], DT_MM, tag="r2", name=f"r2_{b}_{ib}_{i}") for i in range(2)]
                        for h in range(2):
                            for blk in range(8):
                                c = blk * 2 + h
                                pr1 = pvps.tile([1, 128], F32, tag="pv", name=f"pr{b}_{ib}_{c}")
                                nc.tensor.transpose(pr1[:], rT[:, c : c + 1], ident[:])
                                nc.vector.tensor_copy(
                                    r2[h][0:1, blk * 128 : (blk + 1) * 128], pr1[:]
                                )
                        for h in range(2):
                            rbc = pvps.tile([64, 1024], F32, tag="pv")
                            for ih in range(2):
                                nc.tensor.matmul(
                                    rbc[:, ih * 512 : (ih + 1) * 512],
                                    ones1[:],
                                    r2[h][0:1, ih * 512 : (ih + 1) * 512],
                                    start=True,
                                    stop=True,
                                )
                            rbs = sp.tile([64, 1024], F32, tag="rbs")
                            nc.vector.tensor_copy(rbs[:], rbc[:])
                            nc.vector.tensor_tensor(
                                attnT[b][h * 64 : (h + 1) * 64, ib * 1024 : (ib + 1) * 1024],
                                unorm[h][:],
                                rbs[:],
                                mult,
                            )
                        prev_block = (b, ib)

                # flush the last block's out-projection
                for tt in range(8):
                    emit_po(prev_block[0], prev_block[1], tt)


    nc.compile()
    return nc


_NC = None


def _get_nc():
    global _NC
    if _NC is None:
        _NC = build_nc()
    return _NC


def _gate_pm1(mask):
    """Exact jax fp32 gate: sigmoid(m) > 0.5, encoded as +/-1 for device is_gt(0).

    Computed with the same fp32 logistic rounding as the reference (borderline
    tiny-positive m rounds sigmoid to exactly 0.5 -> gate False, unlike m > 0).
    """
    mask = np.asarray(mask, dtype=np.float32)
    g = (np.float32(1.0) / (np.float32(1.0) + np.exp(-mask))) > np.float32(0.5)
    return np.where(g, np.float32(1.0), np.float32(-1.0))


def make_in_maps(x, qkv_weight, qkv_weight_mask, out_weight, out_weight_mask):
    x = np.asarray(x, dtype=np.float32)
    qkv_weight = np.asarray(qkv_weight, dtype=np.float32)
    qkv_weight_mask = _gate_pm1(qkv_weight_mask)
    out_weight = np.asarray(out_weight, dtype=np.float32)
    out_weight_mask = _gate_pm1(out_weight_mask)

    xT = np.ascontiguousarray(x.reshape(T, DIM).T)
    in_maps = []
    for c in range(NCORES):
        r0 = c * DV  # 2c*64
        sl = slice(r0, r0 + DV)
        w_shard = np.concatenate(
            [qkv_weight[sl], qkv_weight[DIM + r0 : DIM + r0 + DV], qkv_weight[2 * DIM + r0 : 2 * DIM + r0 + DV]],
            axis=0,
        )  # [384, 1024] rows = (q h1,h2 | k h1,h2 | v h1,h2)
        m_shard = np.concatenate(
            [
                qkv_weight_mask[sl],
                qkv_weight_mask[DIM + r0 : DIM + r0 + DV],
                qkv_weight_mask[2 * DIM + r0 : 2 * DIM + r0 + DV],
            ],
            axis=0,
        )
        in_maps.append(
            {
                "xT": xT,
                "wqkvT": np.ascontiguousarray(w_shard.T),
                "mqkvT": np.ascontiguousarray(m_shard.T),
                "woT": np.ascontiguousarray(out_weight[:, sl].T),
                "moT": np.ascontiguousarray(out_weight_mask[:, sl].T),
            }
        )
    return in_maps


LAST_RESULTS = None  # BassKernelResults of the most recent run (for profiling)


def kernel(
    x,
    qkv_weight,
    qkv_weight_mask,
    out_weight,
    out_weight_mask,
    out_bias,
    out_bias_mask,
    _trace=False,
    _tmpdir=None,
):
    global LAST_RESULTS
    from concourse.bass_utils import run_bass_kernel_spmd

    nc = _get_nc()
    in_maps = make_in_maps(x, qkv_weight, qkv_weight_mask, out_weight, out_weight_mask)
    res = run_bass_kernel_spmd(
        nc, in_maps, list(range(NCORES)), trace=_trace, tmpdir=_tmpdir
    )
    LAST_RESULTS = res
    out = np.zeros((T, DIM), dtype=np.float32)
    for r in res.results:
        out += r["out"]
    out_bias = np.asarray(out_bias, dtype=np.float32)
    out_bias_mask = np.asarray(out_bias_mask, dtype=np.float32)
    out += np.where(_gate_pm1(out_bias_mask) > 0.0, out_bias, 0.0)[None, :]
    return out.reshape(B, N, DIM)



# revision 5
# speedup vs baseline: 1.3471x; 1.3471x over previous
"""Trainium2 Bass kernel for nn_Attention_41704132444382.

Masked-linear QKV projection + 16-head attention + masked-linear output
projection, tensor-parallel over heads across 8 NeuronCores (2 heads/core).

v2 design notes (vs the 480us baseline):
  - Weights are gated on the HOST (sigmoid(mask)>0.5 applied in numpy) so the
    device never loads masks or runs gating ops.
  - QKV projection runs in float32r (precision: bf16 x/w was the dominant
    error source); everything downstream (S, PV, out-proj) runs in bf16
    (adds ~4e-3 rel err vs the 2e-2 gate).
  - Scores psum tiles are [128, 1024] (2 banks) so ONE exp activation per
    (head, key-tile) covers the whole 1024-token query block: 128 ACTIVATEs
    instead of 256 -> ~25us less ScalarE time.
  - The S -> exp -> PV chain is software-pipelined one key-tile deep:
    per jt the PE queue gets [S(jt) x4, PV(jt-1) x4] so the in-order PE
    never parks on a PV whose exp isn't done; LDWEIGHTS prefetch stays
    hidden behind the previous matmul.
  - Softmax normalization: colsum row (from the ones-column in V) is
    Pool-copied to SBUF, partition_broadcast to 64 partitions, and a single
    DVE tensor_tensor(divide) writes normalized attnT. No PE transposes.
  - Out-projection tiles are pumped into the PE queue at fixed jt slots of
    the NEXT block to fill pipeline bubbles; outputs staged to SBUF as bf16
    partials (host sums in fp32) halving the output DMA.
"""

import os
import sys

import numpy as np

sys.path.insert(0, "/opt/trn_rl_repo")

import concourse.bass as bass
import concourse.mybir as mybir
from concourse import bacc
from concourse.masks import make_identity
from concourse.tile import TileContext

DIM = 1024
HEADS = 16
B = 2
N = 2048
T = B * N  # 4096 flattened tokens
NCORES = 8
HPC = HEADS // NCORES  # 2 heads per core
DV = HPC * 64  # 128 head-dims per core
SCALE = DIM ** (-0.5)  # 1/32

F32 = mybir.dt.float32
F32R = mybir.dt.float32r
BF16 = mybir.dt.bfloat16


def build_nc():
    nc = bacc.Bacc("TRN2", target_bir_lowering=True)
    xT_d = nc.declare_dram_parameter("xT", [DIM, T], F32, isOutput=False)
    wqkvT_d = nc.declare_dram_parameter("wqkvT", [DIM, 384], F32, isOutput=False)
    woT_d = nc.declare_dram_parameter("woT", [DV, DIM], F32, isOutput=False)
    out_d = nc.declare_dram_parameter("out", [T, DIM], BF16, isOutput=True)

    mult = mybir.AluOpType.mult
    divide = mybir.AluOpType.divide
    Exp = mybir.ActivationFunctionType.Exp

    with TileContext(nc) as tc:
        with tc.tile_pool(name="persist", bufs=1) as pp:
            wqkv_g = pp.tile([128, 8 * 384], F32R)  # [k-part, (kt, o)]
            wo_g = pp.tile([128, 1024], BF16)
            qT = pp.tile([128, 4096], BF16)
            kTt = pp.tile([128, 4096], BF16)
            vT = pp.tile([128, 4096], BF16)
            # V with ones column: [t-part, (h, jt32, dv64|1)]
            vv = pp.tile([128, 2 * 32 * 65], BF16)
            attnT = [pp.tile([128, 2048], BF16, name=f"attnT{bb}") for bb in range(B)]
            identf = pp.tile([128, 128], F32)
            identb = pp.tile([128, 128], BF16)

            make_identity(nc, identf[:])
            nc.vector.tensor_copy(identb[:], identf[:])
            ones_f = pp.tile([128, 64], F32)
            nc.vector.memset(ones_f[:], 1.0)
            vv_v = vv[:].rearrange("p (v j c) -> p v j c", v=2, c=65)
            nc.gpsimd.tensor_copy(
                vv_v[:, :, :, 64:65],
                ones_f[:].rearrange("p (v j c) -> p v j c", v=2, c=1),
            )

            # ---------- Phase 0: load pre-gated weights ----------
            with tc.tile_pool(name="wload", bufs=2) as wl:
                wraw = wl.tile([128, 8 * 384], F32, tag="w")
                nc.sync.dma_start(
                    wraw[:].rearrange("p (kt o) -> p kt o", kt=8),
                    wqkvT_d[:].rearrange("(kt p) o -> p kt o", p=128),
                )
                # cast f32 -> f32r in two chunks on two engines for pipelining
                half = 4 * 384
                nc.vector.tensor_copy(wqkv_g[:, :half], wraw[:, :half])
                nc.gpsimd.tensor_copy(wqkv_g[:, half:], wraw[:, half:])

                wor = wl.tile([128, 1024], F32, tag="wo")
                nc.sync.dma_start(wor[:], woT_d[:])
                nc.gpsimd.tensor_copy(wo_g[:], wor[:])

            # ---------- Phase 1: QKV projection ----------
            with (
                tc.tile_pool(name="xq", bufs=16) as xp,
                tc.tile_pool(name="qk_ps", bufs=4, space="PSUM") as qkps,
            ):
                for q in range(4):  # t-quarters of 1024
                    xq = [
                        xp.tile([128, 1024], F32R, tag="xq", name=f"xq{q}_{i}")
                        for i in range(8)
                    ]
                    for kt in range(8):
                        nc.gpsimd.dma_start(
                            xq[kt][:],
                            xT_d[kt * 128 : (kt + 1) * 128, q * 1024 : (q + 1) * 1024],
                        )
                    for ot, dest in enumerate((qT, kTt, vT)):
                        for th in range(2):  # 512-wide halves of the quarter
                            ps = qkps.tile([128, 512], F32, tag="qkps")
                            for kt in range(8):
                                nc.tensor.matmul(
                                    ps[:],
                                    wqkv_g[
                                        :,
                                        kt * 384 + ot * 128 : kt * 384 + (ot + 1) * 128,
                                    ],
                                    xq[kt][:, th * 512 : (th + 1) * 512],
                                    start=(kt == 0),
                                    stop=(kt == 7),
                                )
                            col = q * 1024 + th * 512
                            if (ot * 2 + th) % 2 == 0:
                                nc.vector.tensor_copy(dest[:, col : col + 512], ps[:])
                            else:
                                nc.scalar.copy(dest[:, col : col + 512], ps[:])

            # ---------- Phase 2: attention ----------
            with (
                tc.tile_pool(name="es", bufs=4) as ep,
                tc.tile_pool(name="cs", bufs=4) as csp,
                tc.tile_pool(name="bc", bufs=4) as bcp,
                tc.tile_pool(name="ob", bufs=3) as obp,
                tc.tile_pool(name="s_ps", bufs=2, space="PSUM") as sps,
                tc.tile_pool(name="pv_ps", bufs=2, space="PSUM") as pvps,
            ):
                # V^T [dv, t] -> V [t, dv] via PE transpose; single strided
                # copy scatters both heads into vv's 65-stride layout.
                for jt in range(32):
                    ptv = sps.tile([128, 128], BF16, tag="s", name=f"ptv{jt}")
                    nc.tensor.transpose(
                        ptv[:], vT[:, jt * 128 : (jt + 1) * 128], identb[:]
                    )
                    nc.vector.tensor_copy(
                        vv_v[:, :, jt, 0:64],
                        ptv[:].rearrange("p (v c) -> p v c", v=2),
                    )

                po_queue = []

                def emit_po(pb, pib, tt):
                    tg = pib * 8 + tt
                    po = sps.tile([128, 1024], F32, tag="s", name=f"po{pb}_{pib}_{tt}")
                    for oh in range(2):
                        nc.tensor.matmul(
                            po[:, oh * 512 : (oh + 1) * 512],
                            attnT[pb][:, tg * 128 : (tg + 1) * 128],
                            wo_g[:, oh * 512 : (oh + 1) * 512],
                            start=True,
                            stop=True,
                        )
                    ob = obp.tile([128, 1024], BF16, tag="ob", name=f"ob{pb}_{pib}_{tt}")
                    if tt % 2 == 0:
                        nc.scalar.copy(ob[:], po[:])
                    else:
                        nc.vector.tensor_copy(ob[:], po[:])
                    row = pb * 2048 + tg * 128
                    nc.sync.dma_start(out_d[row : row + 128, :], ob[:])

                def pump(n):
                    for _ in range(min(n, len(po_queue))):
                        emit_po(*po_queue.pop(0))

                for b in range(B):
                    for ib in range(2):  # 1024-wide query blocks
                        i0 = b * 2048 + ib * 1024
                        pv = [
                            pvps.tile([65, 1024], F32, tag="pv", name=f"pv{b}_{ib}_{h}")
                            for h in range(2)
                        ]
                        prev = None  # e tiles of jt-1
                        for jt in range(16):  # 128-wide key tiles
                            j0 = b * 2048 + jt * 128
                            e_h = [
                                ep.tile([128, 1024], BF16, tag="e", name=f"e{b}_{ib}_{jt}_{h}")
                                for h in range(2)
                            ]
                            for h in range(2):
                                st = sps.tile(
                                    [128, 1024], F32, tag="s", name=f"s{b}_{ib}_{jt}_{h}"
                                )
                                kTl = kTt[h * 64 : (h + 1) * 64, j0 : j0 + 128]
                                for ih in range(2):
                                    nc.tensor.matmul(
                                        st[:, ih * 512 : (ih + 1) * 512],
                                        kTl,
                                        qT[
                                            h * 64 : (h + 1) * 64,
                                            i0 + ih * 512 : i0 + (ih + 1) * 512,
                                        ],
                                        start=True,
                                        stop=True,
                                        tile_position=(h * 64, 0),
                                    )
                                nc.scalar.activation(e_h[h][:], st[:], Exp, scale=SCALE)
                            if prev is not None:
                                jv = b * 16 + (jt - 1)
                                for h in range(2):
                                    for ih in range(2):
                                        nc.tensor.matmul(
                                            pv[h][:, ih * 512 : (ih + 1) * 512],
                                            vv_v[:, h, jv, :],
                                            prev[h][:, ih * 512 : (ih + 1) * 512],
                                            start=(jt - 1 == 0),
                                            stop=False,
                                        )
                            prev = e_h
                            if jt == 2:
                                pump(2)
                            elif jt in (4, 6, 8, 10, 12, 14):
                                pump(1)
                        # flush PV for jt=15
                        jv = b * 16 + 15
                        for h in range(2):
                            for ih in range(2):
                                nc.tensor.matmul(
                                    pv[h][:, ih * 512 : (ih + 1) * 512],
                                    vv_v[:, h, jv, :],
                                    prev[h][:, ih * 512 : (ih + 1) * 512],
                                    start=False,
                                    stop=True,
                                )
                        # --- normalization: attnT = pv[0:64] / bcast(colsum) ---
                        for h in range(2):
                            cs = csp.tile([1, 1024], F32, tag="cs", name=f"cs{b}_{ib}_{h}")
                            nc.vector.tensor_copy(cs[:], pv[h][64:65, :])
                            rc = csp.tile([1, 1024], F32, tag="rc", name=f"rc{b}_{ib}_{h}")
                            nc.vector.reciprocal(rc[:], cs[:])
                            bc = bcp.tile([64, 1024], F32, tag="bc", name=f"bc{b}_{ib}_{h}")
                            nc.gpsimd.partition_broadcast(bc[:], rc[:])
                            nc.vector.tensor_tensor(
                                attnT[b][
                                    h * 64 : (h + 1) * 64, ib * 1024 : (ib + 1) * 1024
                                ],
                                pv[h][0:64, :],
                                bc[:],
                                mult,
                            )
                        po_queue.extend((b, ib, tt) for tt in range(8))

                pump(len(po_queue))

    nc.compile()
    return nc


_NC = None


def _get_nc():
    global _NC
    if _NC is None:
        _NC = build_nc()
    return _NC


def _gate(mask):
    """Exact jax fp32 gate: sigmoid(m) > 0.5 (fp32 logistic rounding)."""
    mask = np.asarray(mask, dtype=np.float32)
    return (np.float32(1.0) / (np.float32(1.0) + np.exp(-mask))) > np.float32(0.5)


def make_in_maps(x, qkv_weight, qkv_weight_mask, out_weight, out_weight_mask):
    x = np.asarray(x, dtype=np.float32)
    wq = np.asarray(qkv_weight, dtype=np.float32) * _gate(qkv_weight_mask)
    wo = np.asarray(out_weight, dtype=np.float32) * _gate(out_weight_mask)

    xT = np.ascontiguousarray(x.reshape(T, DIM).T)
    in_maps = []
    for c in range(NCORES):
        r0 = c * DV
        sl = slice(r0, r0 + DV)
        w_shard = np.concatenate(
            [wq[sl], wq[DIM + r0 : DIM + r0 + DV], wq[2 * DIM + r0 : 2 * DIM + r0 + DV]],
            axis=0,
        )  # [384, 1024] rows = (q h1,h2 | k h1,h2 | v h1,h2)
        in_maps.append(
            {
                "xT": xT,
                "wqkvT": np.ascontiguousarray(w_shard.T),
                "woT": np.ascontiguousarray(wo[:, sl].T),
            }
        )
    return in_maps


LAST_RESULTS = None  # BassKernelResults of the most recent run (for profiling)


def kernel(
    x,
    qkv_weight,
    qkv_weight_mask,
    out_weight,
    out_weight_mask,
    out_bias,
    out_bias_mask,
    _trace=False,
    _tmpdir=None,
):
    global LAST_RESULTS
    from concourse.bass_utils import run_bass_kernel_spmd

    nc = _get_nc()
    in_maps = make_in_maps(x, qkv_weight, qkv_weight_mask, out_weight, out_weight_mask)
    res = run_bass_kernel_spmd(
        nc, in_maps, list(range(NCORES)), trace=_trace, tmpdir=_tmpdir
    )
    LAST_RESULTS = res
    out = np.zeros((T, DIM), dtype=np.float32)
    for r in res.results:
        out += np.asarray(r["out"]).astype(np.float32)
    out_bias = np.asarray(out_bias, dtype=np.float32)
    out += np.where(_gate(out_bias_mask), out_bias, np.float32(0.0))[None, :]
    return out.reshape(B, N, DIM)


# revision 9
# speedup vs baseline: 1.4428x; 1.0710x over previous
"""Trainium2 Bass kernel for nn_Attention_41704132444382.

Masked-linear QKV projection + 16-head attention + masked-linear output
projection, tensor-parallel over heads across 8 NeuronCores (2 heads/core).

v2 design notes (vs the 480us baseline):
  - Weights are gated on the HOST (sigmoid(mask)>0.5 applied in numpy) so the
    device never loads masks or runs gating ops.
  - QKV projection runs in float32r (precision: bf16 x/w was the dominant
    error source); everything downstream (S, PV, out-proj) runs in bf16
    (adds ~4e-3 rel err vs the 2e-2 gate).
  - Scores psum tiles are [128, 1024] (2 banks) so ONE exp activation per
    (head, key-tile) covers the whole 1024-token query block: 128 ACTIVATEs
    instead of 256 -> ~25us less ScalarE time.
  - The S -> exp -> PV chain is software-pipelined one key-tile deep:
    per jt the PE queue gets [S(jt) x4, PV(jt-1) x4] so the in-order PE
    never parks on a PV whose exp isn't done; LDWEIGHTS prefetch stays
    hidden behind the previous matmul.
  - Softmax normalization: colsum row (from the ones-column in V) is
    Pool-copied to SBUF, partition_broadcast to 64 partitions, and a single
    DVE tensor_tensor(divide) writes normalized attnT. No PE transposes.
  - Out-projection tiles are pumped into the PE queue at fixed jt slots of
    the NEXT block to fill pipeline bubbles; outputs staged to SBUF as bf16
    partials (host sums in fp32) halving the output DMA.
"""

import os
import sys

import numpy as np

sys.path.insert(0, "/opt/trn_rl_repo")

import concourse.bass as bass
import concourse.mybir as mybir
from concourse import bacc
from concourse.masks import make_identity
from concourse.tile import TileContext

DIM = 1024
HEADS = 16
B = 2
N = 2048
T = B * N  # 4096 flattened tokens
NCORES = 8
HPC = HEADS // NCORES  # 2 heads per core
DV = HPC * 64  # 128 head-dims per core
SCALE = DIM ** (-0.5)  # 1/32

F32 = mybir.dt.float32
F32R = mybir.dt.float32r
BF16 = mybir.dt.bfloat16


def build_nc():
    nc = bacc.Bacc("TRN2", target_bir_lowering=True)
    # declared f32r (same bytes as f32) so hwdge queues can DMA them without
    # the gpsimd-only cast path
    xT_d = nc.declare_dram_parameter("xT", [DIM, T], F32R, isOutput=False)
    wqkvT_d = nc.declare_dram_parameter("wqkvT", [DIM, 384], F32R, isOutput=False)
    woT_d = nc.declare_dram_parameter("woT", [DV, DIM], F32, isOutput=False)
    out_d = nc.declare_dram_parameter("out", [T, DIM], BF16, isOutput=True)

    mult = mybir.AluOpType.mult
    divide = mybir.AluOpType.divide
    Exp = mybir.ActivationFunctionType.Exp

    with TileContext(nc) as tc:
        with tc.tile_pool(name="persist", bufs=1) as pp:
            wqkv_g = pp.tile([128, 8 * 384], F32R)  # [k-part, (kt, o)]
            wo_g = pp.tile([128, 1024], BF16)
            qT = pp.tile([128, 4096], BF16)
            kTt = pp.tile([128, 4096], BF16)
            vT = pp.tile([128, 4096], BF16)
            # V with ones column: [t-part, (h, jt32, dv64|1)]
            vv = pp.tile([128, 2 * 32 * 65], BF16)
            attnT = [pp.tile([128, 2048], BF16, name=f"attnT{bb}") for bb in range(B)]
            identf = pp.tile([128, 128], F32)
            identb = pp.tile([128, 128], BF16)

            make_identity(nc, identf[:])
            nc.vector.tensor_copy(identb[:], identf[:])
            ones_f = pp.tile([128, 64], F32)
            nc.vector.memset(ones_f[:], 1.0)
            vv_v = vv[:].rearrange("p (v j c) -> p v j c", v=2, c=65)
            nc.vector.tensor_copy(
                vv_v[:, :, :, 64:65],
                ones_f[:].rearrange("p (v j c) -> p v j c", v=2, c=1),
            )

            # ---------- Phase 0: load pre-gated weights ----------
            # wqkv loads straight into the f32r tile (same bytes as f32);
            # wo needs a real cast to bf16 (DVE, not urgent).
            with tc.tile_pool(name="wload", bufs=2) as wl:
                nc.sync.dma_start(
                    wqkv_g[:].rearrange("p (kt o) -> p kt o", kt=8),
                    wqkvT_d[:].rearrange("(kt p) o -> p kt o", p=128),
                )
                wor = wl.tile([128, 1024], F32, tag="wo")
                nc.sync.dma_start(wor[:], woT_d[:])
                nc.vector.tensor_copy(wo_g[:], wor[:])

            # ---------- Phase 1: QKV projection ----------
            with (
                tc.tile_pool(name="xq", bufs=16) as xp,
                tc.tile_pool(name="qk_ps", bufs=4, space="PSUM") as qkps,
            ):
                for q in range(4):  # t-quarters of 1024
                    xq = [
                        xp.tile([128, 1024], F32R, tag="xq", name=f"xq{q}_{i}")
                        for i in range(8)
                    ]
                    for kt in range(8):
                        dma_eng = nc.gpsimd if kt % 2 == 0 else nc.sync
                        dma_eng.dma_start(
                            xq[kt][:],
                            xT_d[kt * 128 : (kt + 1) * 128, q * 1024 : (q + 1) * 1024],
                        )
                    for ot, dest in enumerate((qT, kTt, vT)):
                        for th in range(2):  # 512-wide halves of the quarter
                            ps = qkps.tile([128, 512], F32, tag="qkps")
                            for kt in range(8):
                                nc.tensor.matmul(
                                    ps[:],
                                    wqkv_g[
                                        :,
                                        kt * 384 + ot * 128 : kt * 384 + (ot + 1) * 128,
                                    ],
                                    xq[kt][:, th * 512 : (th + 1) * 512],
                                    start=(kt == 0),
                                    stop=(kt == 7),
                                )
                            col = q * 1024 + th * 512
                            if (ot * 2 + th) % 2 == 0:
                                nc.vector.tensor_copy(dest[:, col : col + 512], ps[:])
                            else:
                                nc.scalar.copy(dest[:, col : col + 512], ps[:])

            # ---------- Phase 2: attention ----------
            with (
                tc.tile_pool(name="es", bufs=4) as ep,
                tc.tile_pool(name="cs", bufs=4) as csp,
                tc.tile_pool(name="bc", bufs=4) as bcp,
                tc.tile_pool(name="ob", bufs=3) as obp,
                tc.tile_pool(name="s_ps", bufs=2, space="PSUM") as sps,
                tc.tile_pool(name="pv_ps", bufs=2, space="PSUM") as pvps,
            ):
                # V^T [dv, t] -> V [t, dv] via PE transpose; single strided
                # copy scatters both heads into vv's 65-stride layout.
                for jt in range(32):
                    ptv = sps.tile([128, 128], BF16, tag="s", name=f"ptv{jt}")
                    nc.tensor.transpose(
                        ptv[:], vT[:, jt * 128 : (jt + 1) * 128], identb[:]
                    )
                    nc.vector.tensor_copy(
                        vv_v[:, :, jt, 0:64],
                        ptv[:].rearrange("p (v c) -> p v c", v=2),
                    )

                po_queue = []

                def emit_po(pb, pib, tt):
                    tg = pib * 8 + tt
                    po = sps.tile([128, 1024], F32, tag="s", name=f"po{pb}_{pib}_{tt}")
                    for oh in range(2):
                        nc.tensor.matmul(
                            po[:, oh * 512 : (oh + 1) * 512],
                            attnT[pb][:, tg * 128 : (tg + 1) * 128],
                            wo_g[:, oh * 512 : (oh + 1) * 512],
                            start=True,
                            stop=True,
                        )
                    ob = obp.tile([128, 1024], BF16, tag="ob", name=f"ob{pb}_{pib}_{tt}")
                    if tt % 2 == 0:
                        nc.scalar.copy(ob[:], po[:])
                    else:
                        nc.vector.tensor_copy(ob[:], po[:])
                    row = pb * 2048 + tg * 128
                    nc.sync.dma_start(out_d[row : row + 128, :], ob[:])

                def pump(n):
                    for _ in range(min(n, len(po_queue))):
                        emit_po(*po_queue.pop(0))

                for b in range(B):
                    for ib in range(2):  # 1024-wide query blocks
                        i0 = b * 2048 + ib * 1024
                        pv = [
                            pvps.tile([65, 1024], F32, tag="pv", name=f"pv{b}_{ib}_{h}")
                            for h in range(2)
                        ]
                        prev = None  # e tiles of jt-1
                        for jt in range(16):  # 128-wide key tiles
                            j0 = b * 2048 + jt * 128
                            e_h = [
                                ep.tile([128, 1024], BF16, tag="e", name=f"e{b}_{ib}_{jt}_{h}")
                                for h in range(2)
                            ]
                            for h in range(2):
                                st = sps.tile(
                                    [128, 1024], F32, tag="s", name=f"s{b}_{ib}_{jt}_{h}"
                                )
                                kTl = kTt[h * 64 : (h + 1) * 64, j0 : j0 + 128]
                                for ih in range(2):
                                    nc.tensor.matmul(
                                        st[:, ih * 512 : (ih + 1) * 512],
                                        kTl,
                                        qT[
                                            h * 64 : (h + 1) * 64,
                                            i0 + ih * 512 : i0 + (ih + 1) * 512,
                                        ],
                                        start=True,
                                        stop=True,
                                        tile_position=(h * 64, 0),
                                    )
                                nc.scalar.activation(e_h[h][:], st[:], Exp, scale=SCALE)
                            if prev is not None:
                                jv = b * 16 + (jt - 1)
                                for h in range(2):
                                    for ih in range(2):
                                        nc.tensor.matmul(
                                            pv[h][:, ih * 512 : (ih + 1) * 512],
                                            vv_v[:, h, jv, :],
                                            prev[h][:, ih * 512 : (ih + 1) * 512],
                                            start=(jt - 1 == 0),
                                            stop=False,
                                        )
                            prev = e_h
                            if jt in (6, 8, 14):
                                pump(2)
                            elif jt in (10, 12):
                                pump(1)
                        # flush PV for jt=15
                        jv = b * 16 + 15
                        for h in range(2):
                            for ih in range(2):
                                nc.tensor.matmul(
                                    pv[h][:, ih * 512 : (ih + 1) * 512],
                                    vv_v[:, h, jv, :],
                                    prev[h][:, ih * 512 : (ih + 1) * 512],
                                    start=False,
                                    stop=True,
                                )
                        # --- normalization (lazy): evacuate pv to SBUF fast
                        # (frees the psum accumulators), then recip/broadcast/
                        # mult run in the background off the critical path.
                        for h in range(2):
                            pvs = csp.tile(
                                [65, 1024], F32, tag="pvs", name=f"pvs{b}_{ib}_{h}"
                            )
                            nc.vector.tensor_copy(pvs[:], pv[h][:])
                            rc = csp.tile([1, 1024], F32, tag="rc", name=f"rc{b}_{ib}_{h}")
                            nc.vector.reciprocal(rc[:], pvs[64:65, :])
                            bc = bcp.tile([64, 1024], F32, tag="bc", name=f"bc{b}_{ib}_{h}")
                            nc.gpsimd.partition_broadcast(bc[:], rc[:])
                            nc.vector.tensor_tensor(
                                attnT[b][
                                    h * 64 : (h + 1) * 64, ib * 1024 : (ib + 1) * 1024
                                ],
                                pvs[0:64, :],
                                bc[:],
                                mult,
                            )
                        po_queue.extend((b, ib, tt) for tt in range(8))

                pump(len(po_queue))

    nc.compile()
    return nc


_NC = None


def _get_nc():
    global _NC
    if _NC is None:
        _NC = build_nc()
    return _NC


def _gate(mask):
    """Exact jax fp32 gate: sigmoid(m) > 0.5 (fp32 logistic rounding)."""
    mask = np.asarray(mask, dtype=np.float32)
    return (np.float32(1.0) / (np.float32(1.0) + np.exp(-mask))) > np.float32(0.5)


def make_in_maps(x, qkv_weight, qkv_weight_mask, out_weight, out_weight_mask):
    x = np.asarray(x, dtype=np.float32)
    wq = np.asarray(qkv_weight, dtype=np.float32) * _gate(qkv_weight_mask)
    wo = np.asarray(out_weight, dtype=np.float32) * _gate(out_weight_mask)

    xT = np.ascontiguousarray(x.reshape(T, DIM).T)
    in_maps = []
    for c in range(NCORES):
        r0 = c * DV
        sl = slice(r0, r0 + DV)
        w_shard = np.concatenate(
            [wq[sl], wq[DIM + r0 : DIM + r0 + DV], wq[2 * DIM + r0 : 2 * DIM + r0 + DV]],
            axis=0,
        )  # [384, 1024] rows = (q h1,h2 | k h1,h2 | v h1,h2)
        in_maps.append(
            {
                "xT": xT,
                "wqkvT": np.ascontiguousarray(w_shard.T),
                "woT": np.ascontiguousarray(wo[:, sl].T),
            }
        )
    return in_maps


LAST_RESULTS = None  # BassKernelResults of the most recent run (for profiling)


def kernel(
    x,
    qkv_weight,
    qkv_weight_mask,
    out_weight,
    out_weight_mask,
    out_bias,
    out_bias_mask,
    _trace=False,
    _tmpdir=None,
):
    global LAST_RESULTS
    from concourse.bass_utils import run_bass_kernel_spmd

    nc = _get_nc()
    in_maps = make_in_maps(x, qkv_weight, qkv_weight_mask, out_weight, out_weight_mask)
    res = run_bass_kernel_spmd(
        nc, in_maps, list(range(NCORES)), trace=_trace, tmpdir=_tmpdir
    )
    LAST_RESULTS = res
    out = np.zeros((T, DIM), dtype=np.float32)
    for r in res.results:
        out += np.asarray(r["out"]).astype(np.float32)
    out_bias = np.asarray(out_bias, dtype=np.float32)
    out += np.where(_gate(out_bias_mask), out_bias, np.float32(0.0))[None, :]
    return out.reshape(B, N, DIM)


# revision 14
# speedup vs baseline: 1.5496x; 1.0740x over previous
"""Trainium2 Bass kernel for nn_Attention_41704132444382.

Masked-linear QKV projection + 16-head attention + masked-linear output
projection, tensor-parallel over heads across 8 NeuronCores (2 heads/core).

v3: fully fused single-loop design. The PE clock on TRN2 ramps to 2.4GHz
only under sustained back-to-back work and drops to 1.2GHz whenever the
queue gaps; a bare attention loop is exp-activation-bound with ~0.4us PE
idle per key-tile, which pins the clock at half speed. So everything that
is not the S->exp->PV chain is turned into schedulable PE filler injected
into specific key-tile slots:

  - QKV projection chains for batch 1 run inside batch 0's attention.
  - V transposes, out-projection tiles, and the softmax-normalization
    transposes are likewise spread into exp-bound stretches.
  - Scores psum tiles are [128,1024] (one exp per head per key-tile);
    PV runs one key-tile behind S so the in-order PE queue never parks.
  - Normalization: colsum row -> 16 tiny PE transposes -> reciprocal on
    [128,16] (partition-major: ~100ns vs 6.4us row-major) -> PE transpose
    back -> partition_broadcast -> one DVE mult. All emitted lazily one
    block later; psum accumulators are freed by an immediate [65,1024]
    SBUF evacuation.
  - QKV stays float32r (precision); S/PV/out-proj in bf16 (~7e-3 rel err
    vs the 2e-2 gate). Host gates the masked weights and sums bf16
    partial outputs; f32r dram params let hwdge queues DMA without casts.
"""

import os
import sys

import numpy as np

sys.path.insert(0, "/opt/trn_rl_repo")

import concourse.bass as bass
import concourse.mybir as mybir
from concourse import bacc
from concourse.masks import make_identity
from concourse.tile import TileContext

DIM = 1024
HEADS = 16
B = 2
N = 2048
T = B * N  # 4096 flattened tokens
NCORES = 8
HPC = HEADS // NCORES  # 2 heads per core
DV = HPC * 64  # 128 head-dims per core
SCALE = DIM ** (-0.5)  # 1/32

F32 = mybir.dt.float32
F32R = mybir.dt.float32r
BF16 = mybir.dt.bfloat16


def build_nc():
    nc = bacc.Bacc("TRN2", target_bir_lowering=True)
    # f32r dram params share bytes with f32 but let hwdge queues DMA them
    xT_d = nc.declare_dram_parameter("xT", [DIM, T], F32R, isOutput=False)
    wqkvT_d = nc.declare_dram_parameter("wqkvT", [DIM, 384], F32R, isOutput=False)
    woT_d = nc.declare_dram_parameter("woT", [DV, DIM], F32, isOutput=False)
    out_d = nc.declare_dram_parameter("out", [T, DIM], BF16, isOutput=True)

    mult = mybir.AluOpType.mult
    Exp = mybir.ActivationFunctionType.Exp

    with TileContext(nc) as tc:
        with (
            tc.tile_pool(name="persist", bufs=1) as pp,
            tc.tile_pool(name="xq", bufs=22) as xp,
            tc.tile_pool(name="es", bufs=4) as ep,
            tc.tile_pool(name="cs", bufs=2) as csp,
            tc.tile_pool(name="bc", bufs=1) as bcp,
            tc.tile_pool(name="ob", bufs=3) as obp,
            tc.tile_pool(name="s_ps", bufs=2, space="PSUM") as sps,
            tc.tile_pool(name="pv_ps", bufs=2, space="PSUM") as pvps,
        ):
            wqkv_g = pp.tile([128, 8 * 384], F32R)  # [k-part, (kt, o)]
            wo_g = pp.tile([128, 1024], BF16)
            qT = pp.tile([128, 4096], BF16)
            kTt = pp.tile([128, 4096], BF16)
            vT = pp.tile([128, 4096], BF16)
            # V with ones column: [t-part, (h, jt32, dv64|1)]
            vv = pp.tile([128, 2 * 32 * 65], BF16)
            attnT = [pp.tile([128, 2048], BF16, name=f"attnT{bb}") for bb in range(B)]
            identf = pp.tile([128, 128], F32)
            identb = pp.tile([128, 128], BF16)

            make_identity(nc, identf[:])
            nc.vector.tensor_copy(identb[:], identf[:])
            ones_f = pp.tile([128, 64], F32)
            nc.vector.memset(ones_f[:], 1.0)
            vv_v = vv[:].rearrange("p (v j c) -> p v j c", v=2, c=65)
            nc.vector.tensor_copy(
                vv_v[:, :, :, 64:65],
                ones_f[:].rearrange("p (v j c) -> p v j c", v=2, c=1),
            )

            # ---------- upfront DMAs ----------
            # wqkv per-kt so the first chain starts as soon as kt=0 lands
            for kt in range(8):
                nc.sync.dma_start(
                    wqkv_g[:, kt * 384 : (kt + 1) * 384],
                    wqkvT_d[kt * 128 : (kt + 1) * 128, :],
                )
            wor = pp.tile([128, 1024], F32)
            nc.sync.dma_start(wor[:], woT_d[:])
            nc.vector.tensor_copy(wo_g[:], wor[:])

            xqs = []
            for q in range(4):
                xq = [
                    xp.tile([128, 1024], F32R, tag="xq", name=f"xq{q}_{i}")
                    for i in range(8)
                ]
                for kt in range(8):
                    nc.gpsimd.dma_start(
                        xq[kt][:],
                        xT_d[kt * 128 : (kt + 1) * 128, q * 1024 : (q + 1) * 1024],
                    )
                xqs.append(xq)

            # ---------- emit helpers ----------
            evac_flip = [0]

            def emit_chain(q, ot, th, eng=None):
                """QKV projection chain: 512 tokens x 128 out-channels."""
                dest = (qT, kTt, vT)[ot]
                ps = sps.tile([128, 512], F32, tag="s", name=f"ch{q}_{ot}_{th}")
                for kt in range(8):
                    nc.tensor.matmul(
                        ps[:],
                        wqkv_g[:, kt * 384 + ot * 128 : kt * 384 + (ot + 1) * 128],
                        xqs[q][kt][:, th * 512 : (th + 1) * 512],
                        start=(kt == 0),
                        stop=(kt == 7),
                    )
                col = q * 1024 + th * 512
                if eng is None:
                    eng = nc.vector if evac_flip[0] % 2 == 0 else nc.scalar
                    evac_flip[0] += 1
                if eng is nc.scalar:
                    nc.scalar.copy(dest[:, col : col + 512], ps[:])
                else:
                    eng.tensor_copy(dest[:, col : col + 512], ps[:])

            def emit_vtrans(jt):
                ptv = sps.tile([128, 128], BF16, tag="s", name=f"ptv{jt}")
                nc.tensor.transpose(ptv[:], vT[:, jt * 128 : (jt + 1) * 128], identb[:])
                nc.vector.tensor_copy(
                    vv_v[:, :, jt, 0:64],
                    ptv[:].rearrange("p (v c) -> p v c", v=2),
                )

            ob_flip = [0]

            def emit_po(pb, pib, tt):
                tg = pib * 8 + tt
                po = sps.tile([128, 1024], F32, tag="s", name=f"po{pb}_{pib}_{tt}")
                for oh in range(2):
                    nc.tensor.matmul(
                        po[:, oh * 512 : (oh + 1) * 512],
                        attnT[pb][:, tg * 128 : (tg + 1) * 128],
                        wo_g[:, oh * 512 : (oh + 1) * 512],
                        start=True,
                        stop=True,
                    )
                ob = obp.tile([128, 1024], BF16, tag="ob", name=f"ob{pb}_{pib}_{tt}")
                if ob_flip[0] % 2 == 0:
                    nc.scalar.copy(ob[:], po[:])
                else:
                    nc.vector.tensor_copy(ob[:], po[:])
                ob_flip[0] += 1
                row = pb * 2048 + tg * 128
                nc.sync.dma_start(out_d[row : row + 128, :], ob[:])

            norm_state = {}

            def emit_pvs(b, ib, pv):
                """Evacuate the pv accumulators (fast, frees psum banks).
                The colsum row goes to a partition-0 tile so the norm's PE
                transposes can read it."""
                pvs = []
                for h in range(2):
                    t = csp.tile([64, 1024], F32, tag="pvs", name=f"pvs{b}_{ib}_{h}")
                    nc.vector.tensor_copy(t[:], pv[h][0:64, :])
                    cs = csp.tile([1, 1024], F32, tag="cs", name=f"cs{b}_{ib}_{h}")
                    nc.vector.tensor_copy(cs[:], pv[h][64:65, :])
                    pvs.append((t, cs))
                norm_state[(b, ib)] = pvs

            def emit_norm(b, ib):
                """Lazy background normalization of a finished block.

                colsum rows -> partition-major via PE transposes -> cheap
                reciprocal -> PE transpose back -> partition_broadcast ->
                one mult per head writing normalized attnT."""
                pvs = norm_state.pop((b, ib))
                ptp = sps.tile([128, 16], F32, tag="s", name=f"ptp{b}_{ib}")
                for h in range(2):
                    for blk in range(8):
                        c = blk * 2 + h
                        nc.tensor.transpose(
                            ptp[:, c : c + 1],
                            pvs[h][1][0:1, blk * 128 : (blk + 1) * 128],
                            identf[0:1, 0:1],
                        )
                rt = csp.tile([128, 16], F32, tag="rt", name=f"rt{b}_{ib}")
                nc.vector.tensor_copy(rt[:], ptp[:])
                rcp = csp.tile([128, 16], F32, tag="rcp", name=f"rcp{b}_{ib}")
                nc.vector.reciprocal(rcp[:], rt[:])
                r2s = []
                for h in range(2):
                    r2p = sps.tile([1, 1024], F32, tag="s", name=f"r2p{b}_{ib}_{h}")
                    for blk in range(8):
                        c = blk * 2 + h
                        nc.tensor.transpose(
                            r2p[0:1, blk * 128 : (blk + 1) * 128],
                            rcp[:, c : c + 1],
                            identf[:],
                        )
                    r2 = csp.tile([1, 1024], F32, tag="r2", name=f"r2{b}_{ib}_{h}")
                    nc.vector.tensor_copy(r2[:], r2p[:])
                    r2s.append(r2)
                for h in range(2):
                    bc = bcp.tile([64, 1024], F32, tag="bc", name=f"bc{b}_{ib}_{h}")
                    nc.gpsimd.partition_broadcast(bc[:], r2s[h][:])
                    nc.vector.tensor_tensor(
                        attnT[b][h * 64 : (h + 1) * 64, ib * 1024 : (ib + 1) * 1024],
                        pvs[h][0][:],
                        bc[:],
                        mult,
                    )

            # ---------- bootstrap: QKV(b0) + V(b0) transposes ----------
            for q in (0, 1):
                for ot in (1, 2, 0):  # K, V, Q order
                    for th in range(2):
                        emit_chain(q, ot, th)
            for jt in range(16):
                emit_vtrans(jt)

            # ---------- fused attention loop ----------
            # filler[block][jt] = list of callables emitted after that jt
            filler = {bi: {} for bi in range(4)}
            # block 0: QKV chains for batch 1 (K and V; Q deferred)
            b0_sched = [
                (1, 2, 0), (2, 2, 0), (1, 2, 1), (2, 2, 1),
                (1, 3, 0), (2, 3, 0), (1, 3, 1), (2, 3, 1),
            ]
            for i, (ot, q, th) in enumerate(b0_sched):
                jt = 1 + 2 * i  # jts 1,3,5,7,9,11,13,15
                filler[0].setdefault(jt, []).append(
                    lambda q=q, ot=ot, th=th: emit_chain(q, ot, th, eng=nc.vector)
                )
            # block 1: Q(b1,ib0), V(b1) transposes, norm(block0), po(b0,ib0)
            filler[1].setdefault(0, []).append(lambda: emit_chain(2, 0, 0, eng=nc.vector))
            filler[1].setdefault(1, []).append(lambda: emit_chain(2, 0, 1, eng=nc.vector))
            for i in range(16):
                filler[1].setdefault(2 + i // 4, []).append(
                    lambda jt=16 + i: emit_vtrans(jt)
                )
            filler[1].setdefault(6, []).append(lambda: emit_norm(0, 0))
            for tt in range(8):
                filler[1].setdefault(7 + tt, []).append(
                    lambda tt=tt: emit_po(0, 0, tt)
                )
            # block 2: Q(b1,ib1), norm(block1), po(b0,ib1)
            filler[2].setdefault(0, []).append(lambda: emit_chain(3, 0, 0, eng=nc.vector))
            filler[2].setdefault(1, []).append(lambda: emit_chain(3, 0, 1, eng=nc.vector))
            filler[2].setdefault(3, []).append(lambda: emit_norm(0, 1))
            for tt in range(8):
                filler[2].setdefault(5 + tt, []).append(
                    lambda tt=tt: emit_po(0, 1, tt)
                )
            # block 3: norm(block2), po(b1,ib0)
            filler[3].setdefault(1, []).append(lambda: emit_norm(1, 0))
            for tt in range(8):
                filler[3].setdefault(3 + tt, []).append(
                    lambda tt=tt: emit_po(1, 0, tt)
                )

            for bi in range(4):
                b, ib = bi // 2, bi % 2
                i0 = b * 2048 + ib * 1024
                pv = [
                    pvps.tile([65, 1024], F32, tag="pv", name=f"pv{b}_{ib}_{h}")
                    for h in range(2)
                ]
                prev = None
                for jt in range(16):
                    j0 = b * 2048 + jt * 128
                    e_h = [
                        ep.tile([128, 1024], BF16, tag="e", name=f"e{b}_{ib}_{jt}_{h}")
                        for h in range(2)
                    ]
                    for h in range(2):
                        st = sps.tile(
                            [128, 1024], F32, tag="s", name=f"s{b}_{ib}_{jt}_{h}"
                        )
                        kTl = kTt[h * 64 : (h + 1) * 64, j0 : j0 + 128]
                        for ih in range(2):
                            nc.tensor.matmul(
                                st[:, ih * 512 : (ih + 1) * 512],
                                kTl,
                                qT[
                                    h * 64 : (h + 1) * 64,
                                    i0 + ih * 512 : i0 + (ih + 1) * 512,
                                ],
                                start=True,
                                stop=True,
                                tile_position=(h * 64, 0),
                            )
                        nc.scalar.activation(e_h[h][:], st[:], Exp, scale=SCALE)
                    if prev is not None:
                        jv = b * 16 + (jt - 1)
                        for h in range(2):
                            for ih in range(2):
                                nc.tensor.matmul(
                                    pv[h][:, ih * 512 : (ih + 1) * 512],
                                    vv_v[:, h, jv, :],
                                    prev[h][:, ih * 512 : (ih + 1) * 512],
                                    start=(jt - 1 == 0),
                                    stop=False,
                                )
                    prev = e_h
                    for fn in filler[bi].get(jt, ()):
                        fn()
                # flush PV for jt=15
                jv = b * 16 + 15
                for h in range(2):
                    for ih in range(2):
                        nc.tensor.matmul(
                            pv[h][:, ih * 512 : (ih + 1) * 512],
                            vv_v[:, h, jv, :],
                            prev[h][:, ih * 512 : (ih + 1) * 512],
                            start=False,
                            stop=True,
                        )
                emit_pvs(b, ib, pv)

            # ---------- tail: last norm + po ----------
            emit_norm(1, 1)
            for tt in range(8):
                emit_po(1, 1, tt)

    nc.compile()
    return nc


_NC = None


def _get_nc():
    global _NC
    if _NC is None:
        _NC = build_nc()
    return _NC


def _gate(mask):
    """Exact jax fp32 gate: sigmoid(m) > 0.5 (fp32 logistic rounding)."""
    mask = np.asarray(mask, dtype=np.float32)
    return (np.float32(1.0) / (np.float32(1.0) + np.exp(-mask))) > np.float32(0.5)


def make_in_maps(x, qkv_weight, qkv_weight_mask, out_weight, out_weight_mask):
    x = np.asarray(x, dtype=np.float32)
    wq = np.asarray(qkv_weight, dtype=np.float32) * _gate(qkv_weight_mask)
    wo = np.asarray(out_weight, dtype=np.float32) * _gate(out_weight_mask)

    xT = np.ascontiguousarray(x.reshape(T, DIM).T)
    in_maps = []
    for c in range(NCORES):
        r0 = c * DV
        sl = slice(r0, r0 + DV)
        w_shard = np.concatenate(
            [wq[sl], wq[DIM + r0 : DIM + r0 + DV], wq[2 * DIM + r0 : 2 * DIM + r0 + DV]],
            axis=0,
        )  # [384, 1024] rows = (q h1,h2 | k h1,h2 | v h1,h2)
        in_maps.append(
            {
                "xT": xT,
                "wqkvT": np.ascontiguousarray(w_shard.T),
                "woT": np.ascontiguousarray(wo[:, sl].T),
            }
        )
    return in_maps


LAST_RESULTS = None  # BassKernelResults of the most recent run (for profiling)


def kernel(
    x,
    qkv_weight,
    qkv_weight_mask,
    out_weight,
    out_weight_mask,
    out_bias,
    out_bias_mask,
    _trace=False,
    _tmpdir=None,
):
    global LAST_RESULTS
    from concourse.bass_utils import run_bass_kernel_spmd

    nc = _get_nc()
    in_maps = make_in_maps(x, qkv_weight, qkv_weight_mask, out_weight, out_weight_mask)
    res = run_bass_kernel_spmd(
        nc, in_maps, list(range(NCORES)), trace=_trace, tmpdir=_tmpdir
    )
    LAST_RESULTS = res
    out = np.zeros((T, DIM), dtype=np.float32)
    for r in res.results:
        out += np.asarray(r["out"]).astype(np.float32)
    out_bias = np.asarray(out_bias, dtype=np.float32)
    out += np.where(_gate(out_bias_mask), out_bias, np.float32(0.0))[None, :]
    return out.reshape(B, N, DIM)
